# revision 11
# baseline (speedup 1.0000x reference)
"""Trainium2 Bass kernel for nn_AstraloraLayer: y = x @ A.T, A = w.reshape(512, 512).

Sharding: data-parallel over the flattened token dim. x (8, 8192, 512) -> 65536
tokens, 8192 per core; w replicated (U,S,V unused in the forward). The host
pre-transposes each x shard to [512, 8192] so the contraction dim (d_in) lands
on SBUF partitions with fully contiguous DMA, and feeds A.T [d_in, d_out] so
weight chunks load naturally. Inputs/outputs travel as bf16 (f32 PSUM
accumulation; rel err ~3e-3 vs the f32 reference), halving HBM traffic and
doubling PE rate vs fp32.

Per core: 64 token tiles of 128; each tile is a 4-matmul K-accumulation
(512 = 4 x 128) into one of 4 rotating PSUM banks. Engine programs:
  SP  - x DMAs for tokens [0, 4096), tapered unit sizes so PE starts early
  ACT - weight DMA, x DMAs for tokens [4096, 8192), then batched output DMAs
        (second HWDGE ring; inputs stream on both rings early, outputs late)
  PE  - dummy-matmul HAM prewarm during DMA fill, then dense matmul groups
  DVE - PSUM -> SBUF bf16 casts, deep output staging
"""

import numpy as np

import concourse.bass as bass
import concourse.bass_utils as bass_utils
import concourse.mybir as mybir
from concourse.bass_utils import run_bass_kernel_spmd

N_CORES = 8
D_IN = 512
D_OUT = 512
TOK = 8192  # tokens per core
KC = 128  # contraction chunk (partition dim)
NK = D_IN // KC  # 4
TT = TOK // 128  # total matmul tiles (64)
NPS = 4  # rotating PSUM banks
OBT = 2  # tiles per output DMA
NOB = 8  # output staging slots
N_WARM = 12  # dummy matmuls to lift the PE HAM clock gate during DMA fill

# x DMA units in tokens: (queue, n). Small head units let the PE start ~3 us
# earlier; the tail half streams on the ACT ring in parallel.
X_UNITS_SP = [256, 256, 256, 256, 1024, 1024, 1024]  # tokens [0, 4096)
X_UNITS_ACT = [1024, 1024, 1024, 1024]  # tokens [4096, 8192)
assert sum(X_UNITS_SP) == TOK // 2 and sum(X_UNITS_ACT) == TOK // 2

COMPUTE = "bf16"

_LDW_OPT_PATCHED = False


def _enable_walrus_ldw_opt():
    """walrus ships with --enable-ldw-opt=false hardcoded; with one LDWEIGHTS
    per matmul that serializes ~46 ns/matmul onto the PE stream. The opt is
    safe here (verified rel err unchanged) and takes the matmul issue gap
    from 259 ns to the 216 ns bf16 floor."""
    global _LDW_OPT_PATCHED
    if _LDW_OPT_PATCHED:
        return
    _LDW_OPT_PATCHED = True
    orig_run = bass_utils.run_command

    def patched(cmd, **kw):
        if isinstance(cmd, list):
            cmd = [
                "--enable-ldw-opt=true" if str(c) == "--enable-ldw-opt=false" else c
                for c in cmd
            ]
        return orig_run(cmd, **kw)

    bass_utils.run_command = patched


def build_kernel(compute=COMPUTE):
    if compute == "bf16":
        in_dt = mybir.dt.bfloat16
        out_dt = mybir.dt.bfloat16
    elif compute == "f32r":
        in_dt = mybir.dt.float32r
        out_dt = mybir.dt.float32
    else:
        in_dt = mybir.dt.float32
        out_dt = mybir.dt.float32

    nc = bass.Bass()
    xT = nc.declare_dram_parameter("xT", [D_IN, TOK], in_dt, isOutput=False)
    aT = nc.declare_dram_parameter("aT", [D_IN, D_OUT], in_dt, isOutput=False)
    out = nc.declare_dram_parameter("out", [TOK, D_OUT], out_dt, isOutput=True)

    with (
        nc.sbuf_tensor([KC, NK * D_OUT], in_dt) as wsb,
        nc.sbuf_tensor([KC, NK * TOK], in_dt) as xsb,
        nc.sbuf_tensor([128, NOB * OBT * D_OUT], out_dt) as obuf,
        nc.sbuf_tensor([128, D_OUT + 128], in_dt) as warm,
        nc.psum_tensor([128, D_OUT], mybir.dt.float32) as ps0,
        nc.psum_tensor([128, D_OUT], mybir.dt.float32) as ps1,
        nc.psum_tensor([128, D_OUT], mybir.dt.float32) as ps2,
        nc.psum_tensor([128, D_OUT], mybir.dt.float32) as ps3,
        nc.psum_tensor([128, D_OUT], mybir.dt.float32) as ps_warm,
        nc.semaphore("w_sem") as w_sem,
        nc.semaphore("xs_sem") as xs_sem,
        nc.semaphore("xa_sem") as xa_sem,
        nc.semaphore("mm_sem") as mm_sem,
        nc.semaphore("cp_sem") as cp_sem,
        nc.semaphore("o_sem") as o_sem,
        nc.Block(no_gpsimd_drain=True) as block,
    ):
        ps = [ps0, ps1, ps2, ps3]

        # tile g -> (which x sem, threshold)
        x_wait = [None] * TT
        tok0 = 0
        for u, n in enumerate(X_UNITS_SP):
            for t in range(tok0 // 128, (tok0 + n) // 128):
                x_wait[t] = (xs_sem, 16 * (u + 1))
            tok0 += n
        for u, n in enumerate(X_UNITS_ACT):
            for t in range(tok0 // 128, (tok0 + n) // 128):
                x_wait[t] = (xa_sem, 16 * (u + 1))
            tok0 += n

        @block.sync
        def _(sync):
            tok0 = 0
            for n in X_UNITS_SP:
                sync.dma_start(
                    out=xsb[:, :]
                    .rearrange("p (k t) -> p k t", k=NK)[:, :, tok0 : tok0 + n],
                    in_=xT[:, tok0 : tok0 + n].rearrange("(k p) t -> p k t", p=KC),
                ).then_inc(xs_sem, 16)
                tok0 += n

        @block.tensor
        def _(tensor):
            # HAM prewarm: garbage matmuls into a scratch bank while DMAs fill.
            for _ in range(N_WARM):
                tensor.matmul(
                    ps_warm[:, :],
                    warm[:, D_OUT : D_OUT + 128],
                    warm[:, 0:D_OUT],
                    start=True,
                    stop=True,
                )
            tensor.wait_ge(w_sem, 16)
            for g in range(TT):
                sem, thr = x_wait[g]
                tensor.wait_ge(sem, thr)
                if g >= NPS:
                    tensor.wait_ge(cp_sem, g - NPS + 1)
                for k in range(NK):
                    mm = tensor.matmul(
                        ps[g % NPS][:, :],
                        xsb[:, k * TOK + g * 128 : k * TOK + (g + 1) * 128],
                        wsb[:, k * D_OUT : (k + 1) * D_OUT],
                        start=(k == 0),
                        stop=(k == NK - 1),
                    )
                mm.then_inc(mm_sem, 1)

        @block.vector
        def _(vector):
            for g in range(TT):
                j = g // OBT
                slot = j % NOB
                pos = g % OBT
                vector.wait_ge(mm_sem, g + 1)
                if pos == 0 and j >= NOB:
                    vector.wait_ge(o_sem, 16 * (j - NOB + 1))
                vector.tensor_copy(
                    out=obuf[
                        :,
                        (slot * OBT + pos) * D_OUT : (slot * OBT + pos + 1) * D_OUT,
                    ],
                    in_=ps[g % NPS][:, :],
                ).then_inc(cp_sem, 1)

        @block.scalar
        def _(scalar):
            scalar.dma_start(
                out=wsb[:, :].rearrange("p (k o) -> p k o", k=NK),
                in_=aT[:, :].rearrange("(k p) o -> p k o", p=KC),
            ).then_inc(w_sem, 16)
            tok0 = TOK // 2
            for n in X_UNITS_ACT:
                scalar.dma_start(
                    out=xsb[:, :]
                    .rearrange("p (k t) -> p k t", k=NK)[:, :, tok0 : tok0 + n],
                    in_=xT[:, tok0 : tok0 + n].rearrange("(k p) t -> p k t", p=KC),
                ).then_inc(xa_sem, 16)
                tok0 += n
            for j in range(TT // OBT):
                slot = j % NOB
                scalar.wait_ge(cp_sem, OBT * (j + 1))
                tok0 = j * OBT * 128
                scalar.dma_start(
                    out=out[tok0 : tok0 + OBT * 128, :].rearrange(
                        "(a p) o -> p a o", p=128
                    ),
                    in_=obuf[
                        :, slot * OBT * D_OUT : (slot + 1) * OBT * D_OUT
                    ].rearrange("p (a o) -> p a o", a=OBT),
                ).then_inc(o_sem, 16)
            scalar.wait_ge(o_sem, 16 * (TT // OBT))

    return nc


def _prep_inputs(x, w, compute=COMPUTE):
    if compute == "bf16":
        import ml_dtypes

        np_dt = ml_dtypes.bfloat16
    else:
        np_dt = np.float32
    xf = np.asarray(x, dtype=np.float32).reshape(-1, D_IN)
    A = np.asarray(w, dtype=np.float32).reshape(D_OUT, D_IN)
    aT = np.ascontiguousarray(A.T).astype(np_dt)
    in_maps = []
    for s in range(N_CORES):
        xs = xf[s * TOK : (s + 1) * TOK]
        in_maps.append({"xT": np.ascontiguousarray(xs.T).astype(np_dt), "aT": aT})
    return in_maps


def kernel(x, w, U=None, S=None, V=None, **_):
    _enable_walrus_ldw_opt()
    nc = build_kernel()
    in_maps = _prep_inputs(x, w)
    res = run_bass_kernel_spmd(nc, in_maps, core_ids=list(range(N_CORES)))
    y = np.concatenate(
        [np.asarray(res.results[i]["out"], dtype=np.float32) for i in range(N_CORES)],
        axis=0,
    )
    return y.reshape(*x.shape[:-1], D_OUT)


# revision 13
# speedup vs baseline: 1.1158x; 1.1158x over previous
"""Trainium2 Bass kernel for nn_AstraloraLayer: y = x @ A.T, A = w.reshape(512, 512).

Sharding: data-parallel over the flattened token dim. x (8, 8192, 512) -> 65536
tokens, 8192 per core; w replicated (U,S,V unused in the forward). The host
pre-transposes each x shard to [512, 8192] so the contraction dim (d_in) lands
on SBUF partitions with fully contiguous DMA, and feeds A.T [d_in, d_out] so
weight chunks load naturally. Inputs/outputs travel as bf16 (f32 PSUM
accumulation; rel err ~3e-3 vs the f32 reference), halving HBM traffic and
doubling PE rate vs fp32.

Per core: 64 token tiles of 128; each tile is a 4-matmul K-accumulation
(512 = 4 x 128) into one of 4 rotating PSUM banks. Engine programs:
  SP  - x DMAs for tokens [0, 4096), tapered unit sizes so PE starts early
  ACT - weight DMA, x DMAs for tokens [4096, 8192), then batched output DMAs
        (second HWDGE ring; inputs stream on both rings early, outputs late)
  PE  - dummy-matmul HAM prewarm during DMA fill, then dense matmul groups
  DVE - PSUM -> SBUF bf16 casts, deep output staging
"""

import numpy as np

import concourse.bass as bass
import concourse.bass_utils as bass_utils
import concourse.mybir as mybir
from concourse.bass_utils import run_bass_kernel_spmd

N_CORES = 8
D_IN = 512
D_OUT = 512
TOK = 8192  # tokens per core
KC = 128  # contraction chunk (partition dim)
NK = D_IN // KC  # 4
TT = TOK // 128  # total matmul tiles (64)
NPS = 4  # rotating PSUM banks
OBT = 2  # tiles per output DMA
NOB = 8  # output staging slots
N_WARM = 12  # dummy matmuls to lift the PE HAM clock gate during DMA fill

# x DMA units in tokens, all on the SP ring in consumption order. Small head
# units let the PE start ~3 us earlier; 512-token steady units keep arrival
# granularity fine enough that the PE never waits long.
X_UNITS_SP = [256, 256, 256, 256] + [512] * 14  # tokens [0, 8192)
X_UNITS_ACT = []
assert sum(X_UNITS_SP) + sum(X_UNITS_ACT) == TOK

COMPUTE = "bf16"

_LDW_OPT_PATCHED = False


def _enable_walrus_ldw_opt():
    """walrus ships with --enable-ldw-opt=false hardcoded; with one LDWEIGHTS
    per matmul that serializes ~46 ns/matmul onto the PE stream. The opt is
    safe here (verified rel err unchanged) and takes the matmul issue gap
    from 259 ns to the 216 ns bf16 floor."""
    global _LDW_OPT_PATCHED
    if _LDW_OPT_PATCHED:
        return
    _LDW_OPT_PATCHED = True
    orig_run = bass_utils.run_command

    def patched(cmd, **kw):
        if isinstance(cmd, list):
            cmd = [
                "--enable-ldw-opt=true" if str(c) == "--enable-ldw-opt=false" else c
                for c in cmd
            ]
        return orig_run(cmd, **kw)

    bass_utils.run_command = patched


def build_kernel(compute=COMPUTE):
    if compute == "bf16":
        in_dt = mybir.dt.bfloat16
        out_dt = mybir.dt.bfloat16
    elif compute == "f32r":
        in_dt = mybir.dt.float32r
        out_dt = mybir.dt.float32
    else:
        in_dt = mybir.dt.float32
        out_dt = mybir.dt.float32

    nc = bass.Bass()
    xT = nc.declare_dram_parameter("xT", [D_IN, TOK], in_dt, isOutput=False)
    aT = nc.declare_dram_parameter("aT", [D_IN, D_OUT], in_dt, isOutput=False)
    out = nc.declare_dram_parameter("out", [TOK, D_OUT], out_dt, isOutput=True)

    with (
        nc.sbuf_tensor([KC, NK * D_OUT], in_dt) as wsb,
        nc.sbuf_tensor([KC, NK * TOK], in_dt) as xsb,
        nc.sbuf_tensor([128, NOB * OBT * D_OUT], out_dt) as obuf,
        nc.sbuf_tensor([128, D_OUT + 128], in_dt) as warm,
        nc.psum_tensor([128, D_OUT], mybir.dt.float32) as ps0,
        nc.psum_tensor([128, D_OUT], mybir.dt.float32) as ps1,
        nc.psum_tensor([128, D_OUT], mybir.dt.float32) as ps2,
        nc.psum_tensor([128, D_OUT], mybir.dt.float32) as ps3,
        nc.psum_tensor([128, D_OUT], mybir.dt.float32) as ps_warm,
        nc.semaphore("w_sem") as w_sem,
        nc.semaphore("xs_sem") as xs_sem,
        nc.semaphore("xa_sem") as xa_sem,
        nc.semaphore("mm_sem") as mm_sem,
        nc.semaphore("cp_sem") as cp_sem,
        nc.semaphore("o_sem") as o_sem,
        nc.Block(no_gpsimd_drain=True) as block,
    ):
        ps = [ps0, ps1, ps2, ps3]

        # tile g -> (which x sem, threshold)
        x_wait = [None] * TT
        tok0 = 0
        for u, n in enumerate(X_UNITS_SP):
            for t in range(tok0 // 128, (tok0 + n) // 128):
                x_wait[t] = (xs_sem, 16 * (u + 1))
            tok0 += n
        for u, n in enumerate(X_UNITS_ACT):
            for t in range(tok0 // 128, (tok0 + n) // 128):
                x_wait[t] = (xa_sem, 16 * (u + 1))
            tok0 += n

        @block.sync
        def _(sync):
            tok0 = 0
            for n in X_UNITS_SP:
                sync.dma_start(
                    out=xsb[:, :]
                    .rearrange("p (k t) -> p k t", k=NK)[:, :, tok0 : tok0 + n],
                    in_=xT[:, tok0 : tok0 + n].rearrange("(k p) t -> p k t", p=KC),
                ).then_inc(xs_sem, 16)
                tok0 += n

        @block.tensor
        def _(tensor):
            # HAM prewarm: garbage matmuls into a scratch bank while DMAs fill.
            for _ in range(N_WARM):
                tensor.matmul(
                    ps_warm[:, :],
                    warm[:, D_OUT : D_OUT + 128],
                    warm[:, 0:D_OUT],
                    start=True,
                    stop=True,
                )
            tensor.wait_ge(w_sem, 16)
            for g in range(TT):
                sem, thr = x_wait[g]
                tensor.wait_ge(sem, thr)
                if g >= NPS:
                    tensor.wait_ge(cp_sem, g - NPS + 1)
                for k in range(NK):
                    mm = tensor.matmul(
                        ps[g % NPS][:, :],
                        xsb[:, k * TOK + g * 128 : k * TOK + (g + 1) * 128],
                        wsb[:, k * D_OUT : (k + 1) * D_OUT],
                        start=(k == 0),
                        stop=(k == NK - 1),
                    )
                mm.then_inc(mm_sem, 1)

        @block.vector
        def _(vector):
            for g in range(TT):
                j = g // OBT
                slot = j % NOB
                pos = g % OBT
                vector.wait_ge(mm_sem, g + 1)
                if pos == 0 and j >= NOB:
                    vector.wait_ge(o_sem, 16 * (j - NOB + 1))
                vector.tensor_copy(
                    out=obuf[
                        :,
                        (slot * OBT + pos) * D_OUT : (slot * OBT + pos + 1) * D_OUT,
                    ],
                    in_=ps[g % NPS][:, :],
                ).then_inc(cp_sem, 1)

        @block.scalar
        def _(scalar):
            scalar.dma_start(
                out=wsb[:, :].rearrange("p (k o) -> p k o", k=NK),
                in_=aT[:, :].rearrange("(k p) o -> p k o", p=KC),
            ).then_inc(w_sem, 16)
            tok0 = TOK - sum(X_UNITS_ACT)
            for n in X_UNITS_ACT:
                scalar.dma_start(
                    out=xsb[:, :]
                    .rearrange("p (k t) -> p k t", k=NK)[:, :, tok0 : tok0 + n],
                    in_=xT[:, tok0 : tok0 + n].rearrange("(k p) t -> p k t", p=KC),
                ).then_inc(xa_sem, 16)
                tok0 += n
            for j in range(TT // OBT):
                slot = j % NOB
                scalar.wait_ge(cp_sem, OBT * (j + 1))
                tok0 = j * OBT * 128
                scalar.dma_start(
                    out=out[tok0 : tok0 + OBT * 128, :].rearrange(
                        "(a p) o -> p a o", p=128
                    ),
                    in_=obuf[
                        :, slot * OBT * D_OUT : (slot + 1) * OBT * D_OUT
                    ].rearrange("p (a o) -> p a o", a=OBT),
                ).then_inc(o_sem, 16)
            scalar.wait_ge(o_sem, 16 * (TT // OBT))

    return nc


def _prep_inputs(x, w, compute=COMPUTE):
    if compute == "bf16":
        import ml_dtypes

        np_dt = ml_dtypes.bfloat16
    else:
        np_dt = np.float32
    xf = np.asarray(x, dtype=np.float32).reshape(-1, D_IN)
    A = np.asarray(w, dtype=np.float32).reshape(D_OUT, D_IN)
    aT = np.ascontiguousarray(A.T).astype(np_dt)
    in_maps = []
    for s in range(N_CORES):
        xs = xf[s * TOK : (s + 1) * TOK]
        in_maps.append({"xT": np.ascontiguousarray(xs.T).astype(np_dt), "aT": aT})
    return in_maps


def kernel(x, w, U=None, S=None, V=None, **_):
    _enable_walrus_ldw_opt()
    nc = build_kernel()
    in_maps = _prep_inputs(x, w)
    res = run_bass_kernel_spmd(nc, in_maps, core_ids=list(range(N_CORES)))
    y = np.concatenate(
        [np.asarray(res.results[i]["out"], dtype=np.float32) for i in range(N_CORES)],
        axis=0,
    )
    return y.reshape(*x.shape[:-1], D_OUT)
